# revision 1
# baseline (speedup 1.0000x reference)
"""Multi-head attention (B=2, T=4096, D=512, H=8) on 8 Trainium2 cores.

Sharding: core i handles batch b=i//4, query rows q0=(i%4)*1024 .. q0+1024,
all 8 heads (full K/V of its batch computed on-core; no collectives).
Host pre-transposes x and weights so every DMA is contiguous, and rolls
x along T per core so each core's query block sits at columns 0:1024
(keys become a permutation of T, which attention is invariant to).

All matmuls run in float32r (TF32-like single-pass PE mode, ~1.5e-4 rel
err measured on HW). Softmax skips the max-subtraction (scores are
~N(0, 0.33); exp cannot overflow) and the row-sum comes free from a ones
column appended to V in the attn@V matmul (output partition 64).
"""
import sys
sys.path.insert(0, "/opt/trn_rl_repo")

import numpy as np
import concourse.bacc as bacc
import concourse.mybir as mybir
import concourse.tile as tile
from concourse.bass_utils import run_bass_kernel_spmd

F32 = mybir.dt.float32
F32R = mybir.dt.float32r
AF = mybir.ActivationFunctionType
MULT = mybir.AluOpType.mult

B, T, C = 2, 4096, 512
H, DK = 8, 64
TQ = 1024          # queries per core
NP = 4             # head pairs
KT = T // 128      # 32 k-tiles
CT = C // 128      # 4 contraction tiles

_cache = {}


def _build():
    nc = bacc.Bacc("TRN2")
    xbT = nc.declare_dram_parameter("xbT", [C, T], F32R, isOutput=False)
    wqT = nc.declare_dram_parameter("wqT", [C, C], F32R, isOutput=False)
    wkT = nc.declare_dram_parameter("wkT", [C, C], F32R, isOutput=False)
    wvT = nc.declare_dram_parameter("wvT", [C, C], F32R, isOutput=False)
    woT = nc.declare_dram_parameter("woT", [C, C], F32R, isOutput=False)
    # bias[:, 0] = bq/8, bias[:, 1] = bk, bias[:, 2] = bv  (col-block per pair)
    bias = nc.declare_dram_parameter("bias", [128, 3, NP], F32, isOutput=False)
    bo = nc.declare_dram_parameter("bo", [1, C], F32R, isOutput=False)
    # ind rows: 0 = head0 mask (1s in 0:64), 1 = head1 mask, 2 = all ones
    ind = nc.declare_dram_parameter("ind", [3, 128], F32R, isOutput=False)
    ones = nc.declare_dram_parameter("ones", [128, KT * 4], F32R, isOutput=False)
    out = nc.declare_dram_parameter("out", [TQ, C], F32, isOutput=True)

    with tile.TileContext(nc) as tc:
        attn_bufs, kt_bufs, big_bufs = 4, 2, 3
        use_prj, av_single = False, True
        with (
            tc.tile_pool(name="big", bufs=1) as bpool,
            tc.tile_pool(name="const", bufs=1) as cpool,
            tc.tile_pool(name="work", bufs=2) as wpool,
            tc.tile_pool(name="ktp", bufs=kt_bufs) as ktpool,
            tc.tile_pool(name="attnp", bufs=attn_bufs) as apool,
            tc.tile_pool(name="ps", bufs=big_bufs, space="PSUM") as ps,
            tc.tile_pool(name="prj", bufs=1, space="PSUM") as _psprj,
            tc.tile_pool(name="psav", bufs=1, space="PSUM") as psav,
        ):
            psprj = _psprj if use_prj else ps
            prjtag = "proj" if use_prj else "big"
            # ---- resident tensors ----
            xT = bpool.tile([128, CT, T], F32R, tag="xT")          # 64KB/part
            for ct in range(CT):
                for tch in range(4):
                    nc.sync.dma_start(
                        xT[:, ct, tch * 1024:(tch + 1) * 1024],
                        xbT[ct * 128:(ct + 1) * 128, tch * 1024:(tch + 1) * 1024])
            woTs = cpool.tile([128, CT, C], F32R, tag="woT")       # 8KB
            for ct in range(CT):
                nc.sync.dma_start(woTs[:, ct, :], woT[ct * 128:(ct + 1) * 128, :])
            bias_s = cpool.tile([128, 3, NP], F32, tag="bias")
            nc.sync.dma_start(bias_s[:], bias[:])
            # ind / bo live at partition 64 so matmul operand bases match the
            # rowsum row (PSUM partition 64) they pair with.
            inds = cpool.tile([65, 3, 128], F32R, tag="ind")
            nc.sync.dma_start(inds[64:65, :, :],
                              ind.rearrange("(o a) b -> o a b", o=1))
            bos = cpool.tile([65, C], F32R, tag="bo")
            nc.sync.dma_start(bos[64:65, :], bo[:])
            acat = bpool.tile([128, NP, TQ], F32R, tag="acat")     # 16KB

            # ---- V projection for one pair-group (2 pairs = 4 heads) ----
            # v2p[:, j*4 + pi*2 + h, 0:64] = V rows, col 64 = ones
            def v_proj(pg):
                v2p = bpool.tile([128, KT * 4, 65], F32R, tag="v2p")  # 33KB
                nc.sync.dma_start(v2p[:, :, 64], ones[:])
                wvs = bpool.tile([128, CT, 256], F32R, tag="wvs")
                for ct in range(CT):
                    nc.sync.dma_start(
                        wvs[:, ct, :],
                        wvT[ct * 128:(ct + 1) * 128, pg * 256:(pg + 1) * 256])
                for j in range(KT):
                    pv = psprj.tile([128, 1024], F32, tag=prjtag)
                    for ct in range(CT):
                        nc.tensor.matmul(
                            pv[:, 0:256],
                            xT[:, ct, j * 128:(j + 1) * 128],
                            wvs[:, ct, :],
                            start=(ct == 0), stop=(ct == CT - 1))
                    nc.vector.tensor_copy(
                        v2p[:, j * 4:(j + 1) * 4, 0:64],
                        pv[:, 0:256].rearrange("p (a b) -> p a b", b=64))
                return v2p

            def projs(p):
                # --- K^T projection: [128 d, 4096 t], bias bk per-partition ---
                kT = ktpool.tile([128, T], F32R, tag="kT")
                wks = bpool.tile([128, CT, 128], F32R, tag="wks")
                for ct in range(CT):
                    nc.sync.dma_start(
                        wks[:, ct, :],
                        wkT[ct * 128:(ct + 1) * 128, p * 128:(p + 1) * 128])
                for tp in range(4):
                    pk = psprj.tile([128, 1024], F32, tag=prjtag)
                    for half in range(2):
                        tch = tp * 2 + half
                        for ct in range(CT):
                            nc.tensor.matmul(
                                pk[:, half * 512:(half + 1) * 512],
                                wks[:, ct, :],
                                xT[:, ct, tch * 512:(tch + 1) * 512],
                                start=(ct == 0), stop=(ct == CT - 1))
                    nc.vector.tensor_scalar_add(
                        kT[:, tp * 1024:(tp + 1) * 1024], pk[:],
                        bias_s[:, 1, p:p + 1])
                # --- Q^T projection: [128 d, 1024 q], scale 1/8, bias bq/8 ---
                qT = ktpool.tile([128, TQ], F32R, tag="qT")
                wqs = bpool.tile([128, CT, 128], F32R, tag="wqs")
                for ct in range(CT):
                    nc.sync.dma_start(
                        wqs[:, ct, :],
                        wqT[ct * 128:(ct + 1) * 128, p * 128:(p + 1) * 128])
                pq = psprj.tile([128, 1024], F32, tag=prjtag)
                for half in range(2):
                    for ct in range(CT):
                        nc.tensor.matmul(
                            pq[:, half * 512:(half + 1) * 512],
                            wqs[:, ct, :],
                            xT[:, ct, half * 512:(half + 1) * 512],
                            start=(ct == 0), stop=(ct == CT - 1))
                nc.scalar.activation(qT[:], pq[:], AF.Identity,
                                     bias=bias_s[:, 0, p:p + 1], scale=0.125)
                return kT, qT

            def attn_pair(p, v2p, pi, kT, qT):
                # --- attention: scores^T -> exp -> attn@[V|1] accumulate ---
                stage = bpool.tile([65, 2, TQ], F32R, tag="stage")   # 8KB
                odd = wpool.tile([64, TQ], F32R, tag="sc")
                for h in range(2):
                    av = psav.tile([65, TQ], F32, tag="av" if av_single else f"av{h}")
                    d0 = h * 64
                    # pipeline unit = 3 half-tiles (1.5 k-tiles) to amortize
                    # the ACT per-instruction overhead; accumulation flags
                    # stay keyed on the k-tile index per 512-col PSUM region
                    Hh = 0
                    while Hh < 2 * KT:
                        n = min(2, 2 * KT - Hh)
                        pss = ps.tile([128, 512 * n], F32, tag="big")
                        for pos in range(n):
                            j, half = (Hh + pos) // 2, (Hh + pos) % 2
                            nc.tensor.matmul(
                                pss[:, pos * 512:(pos + 1) * 512],
                                kT[d0:d0 + 64, j * 128:(j + 1) * 128],
                                qT[d0:d0 + 64, half * 512:(half + 1) * 512],
                                start=True, stop=True)
                        at = apool.tile([128, 512 * n], F32R, tag="attn")
                        nc.scalar.activation(at[:], pss[:], AF.Exp)
                        for pos in range(n):
                            j, half = (Hh + pos) // 2, (Hh + pos) % 2
                            nc.tensor.matmul(
                                av[:, half * 512:(half + 1) * 512],
                                v2p[:, j * 4 + pi * 2 + h, :],
                                at[:, pos * 512:(pos + 1) * 512],
                                start=(j == 0), stop=(j == KT - 1))
                        Hh += n
                    # drain this head's accumulator so the next head can
                    # reuse the single PSUM slot
                    nc.vector.tensor_copy(stage[64:65, h, :], av[64:65, :])
                    if h == 0:
                        nc.vector.tensor_copy(acat[0:64, p, :], av[0:64, :])
                    else:
                        nc.vector.tensor_copy(odd[:], av[0:64, :])
                        nc.sync.dma_start(acat[64:128, p, :], odd[:])

                # --- normalize: U / rowsum + bv  into acat[:, p, :] ---
                pb = psprj.tile([128, 1024], F32, tag=prjtag)
                for half in range(2):
                    for h in range(2):
                        nc.tensor.matmul(
                            pb[:, half * 512:(half + 1) * 512],
                            inds[64:65, h, :],
                            stage[64:65, h, half * 512:(half + 1) * 512],
                            start=(h == 0), stop=(h == 1))
                rb = wpool.tile([128, TQ], F32, tag="sc")
                nc.vector.reciprocal(rb[:], pb[:])
                nc.vector.tensor_tensor(
                    out=acat[:, p, :], in0=acat[:, p, :], in1=rb[:], op=MULT)
                nc.vector.tensor_scalar_add(
                    acat[:, p, :], acat[:, p, :], bias_s[:, 2, p:p + 1])

            kq = projs(0)
            v2p = v_proj(0)
            attn_pair(0, v2p, 0, *kq)
            for p in range(1, NP):
                kq = projs(p)
                if p == 2:
                    v2p = v_proj(1)
                attn_pair(p, v2p, p % 2, *kq)

            # ---- output projection: out[t, :] = acat^T.T @ woT + bo ----
            for qt in range(8):
                po = psprj.tile([128, 1024], F32, tag=prjtag)
                for r in range(CT):
                    nc.tensor.matmul(
                        po[:, 0:512],
                        acat[:, r, qt * 128:(qt + 1) * 128],
                        woTs[:, r, :],
                        start=(r == 0), stop=False)
                nc.tensor.matmul(po[:, 0:512], inds[64:65, 2, :],
                                 bos[64:65, :], start=False, stop=True)
                ot = wpool.tile([128, C], F32, tag="sc")
                nc.vector.tensor_copy(ot[:], po[:, 0:512])
                nc.sync.dma_start(out[qt * 128:(qt + 1) * 128, :], ot[:])

    nc.compile()
    return nc


def _prep_inputs(x, Wq, bq, Wk, bk, Wv, bv, Wo, bo):
    wqT = np.ascontiguousarray(Wq.T)
    wkT = np.ascontiguousarray(Wk.T)
    wvT = np.ascontiguousarray(Wv.T)
    woT = np.ascontiguousarray(Wo.T)
    bias = np.stack([
        (bq / 8.0).reshape(NP, 128).T,
        bk.reshape(NP, 128).T,
        bv.reshape(NP, 128).T,
    ], axis=1).astype(np.float32)          # [128, 3, NP]
    bias = np.ascontiguousarray(bias)
    bo1 = np.ascontiguousarray(bo.reshape(1, C))
    ind = np.zeros((3, 128), np.float32)
    ind[0, 0:64] = 1.0
    ind[1, 64:128] = 1.0
    ind[2, :] = 1.0
    in_maps = []
    for i in range(8):
        b, q0 = i // 4, (i % 4) * TQ
        xbT = np.ascontiguousarray(np.roll(x[b].T, -q0, axis=1))
        in_maps.append({
            "xbT": xbT, "wqT": wqT, "wkT": wkT, "wvT": wvT, "woT": woT,
            "bias": bias, "bo": bo1, "ind": ind,
            "ones": np.ones((128, KT * 4), np.float32),
        })
    return in_maps


def kernel(x, Wq, bq, Wk, bk, Wv, bv, Wo, bo):
    x = np.asarray(x, np.float32)
    args = [np.asarray(a, np.float32) for a in
            (Wq, bq, Wk, bk, Wv, bv, Wo, bo)]
    if "nc" not in _cache:
        _cache["nc"] = _build()
    nc = _cache["nc"]
    in_maps = _prep_inputs(x, *args)
    res = run_bass_kernel_spmd(nc, in_maps, list(range(8)))
    outf = np.empty((B, T, C), np.float32)
    for i in range(8):
        b, q0 = i // 4, (i % 4) * TQ
        outf[b, q0:q0 + TQ, :] = res.results[i]["out"]
    return outf



# revision 25
# speedup vs baseline: 1.2108x; 1.2108x over previous
"""Multi-head attention (B=2, T=4096, D=512, H=8) on 8 Trainium2 cores.

Sharding: core i handles batch b=i//4, query rows q0=(i%4)*1024 .. q0+1024,
all 8 heads (full K/V of its batch computed on-core; no collectives).
Host pre-transposes x and weights so every DMA is contiguous, and rolls
x along T per core so each core's query block sits at columns 0:1024.

Projections run in float32r (TF32-like single-pass PE mode). Attention:
scores^T [128k, 1024q] in f32r; exp split between the Activation engine
(table Exp) and a custom-DVE polynomial (P3(s/32))^4 ~ c*e^(s/8) (softmax
cancels c; ACT tiles carry a matching bias) with output in bf16; attn@V
runs with q on PSUM partitions (out [128q, 65], bf16 moving free 65) so
the ones-column row-sum is a per-partition scalar: normalize with
reciprocal + tensor_scalar, PE-transpose back, and O-project with bv
folded into bo on the host (out = A@Wo^T + (bo + Wo@bv)).

All PSUM [128,1024] f32 tiles (scores, projections, transposes) share one
3-buf pool; V-projection k-tiles are emitted inside the attention loops
(lookahead 10) so the exp pipeline never starves. The scores pipeline
runs 3 k-tiles ahead of attn@V so the in-order PE never waits on an exp."""
import sys
sys.path.insert(0, "/opt/trn_rl_repo")

import math
import numpy as np
import ml_dtypes

import concourse.bacc as bacc
import concourse.mybir as mybir
import concourse.tile as tile
import concourse.dve_ops as dops
from concourse.dve_spec import Spec, Bin, AluOp, Src0, C0, C1, C2, One, sq, lower
from concourse.dve_uop import DveOpSpec
from concourse.bass_utils import run_bass_kernel_spmd

F32 = mybir.dt.float32
F32R = mybir.dt.float32r
BF16 = mybir.dt.bfloat16
AF = mybir.ActivationFunctionType
MULT = mybir.AluOpType.mult
ADD = mybir.AluOpType.add

B, T, C = 2, 4096, 512
H, DK = 8, 64
TQ = 1024          # queries per core
NP = 4             # head pairs
KT = T // 128      # 32 k-tiles
CT = C // 128      # 4 contraction tiles
VLOOK = 10         # v-proj k-tiles emitted this far ahead of attn@V use

# deg-3 fit of e^u on [-0.57, 0.57]; P/a0 has unit constant term and
# approximates (1/a0)*e^u. (P/a0)^4 = c*e^(4u); softmax cancels c, the
# ACT-exp tiles get bias=-4*ln(a0) so both engines agree.
_A0, _A1, _A2, _A3 = 0.99951510, 1.00118603, 0.51320528, 0.16396123
EXP_B1 = (_A1 / _A0) / 32.0
EXP_B2 = (_A2 / _A0) / 32.0**2
EXP_B3 = (_A3 / _A0) / 32.0**3
ACT_EXP_BIAS = -4.0 * math.log(_A0)
DVE_EXP_ON = True


def _register_exp_op():
    name = "EXP_POLY34_ANT"
    if name in dops._SUB_OPCODE_FOR_NAME:
        return next(o for o in dops.OPS if o.name == name)
    body = sq(sq(Bin(AluOp.ADD,
                     Bin(AluOp.MULTIPLY,
                         Bin(AluOp.ADD,
                             Bin(AluOp.MULTIPLY,
                                 Bin(AluOp.ADD, Bin(AluOp.MULTIPLY, C0, Src0), C1),
                                 Src0),
                             C2),
                         Src0),
                     One)))

    def ref(in0, in1, s0, s1, imm2):
        x = in0.astype(np.float32)
        p = ((s0 * x + s1) * x + imm2) * x + 1.0
        return (p * p) * (p * p)

    spec = Spec(body=body, reference=ref)
    row = dops._CUSTOM_DVE_ROW_BASE + len(dops.OPS)
    assert row < 0x20
    dops._SUB_OPCODE_FOR_NAME[name] = row
    sha = DveOpSpec(name=name, opcode=row, uops=lower(spec, ver="v3"),
                    rd1_en=False).sha("v3")
    op = dops.DveOp(name=name, spec=spec, subdim=False, uops_sha={"v3": sha})
    dops.OPS.append(op)
    dops.CUSTOM_DVE_SPECS[name] = spec
    return op


EXP_OP = _register_exp_op()
_cache = {}


def _build():
    nc = bacc.Bacc("TRN2")
    xbT = nc.declare_dram_parameter("xbT", [C, T], F32R, isOutput=False)
    wqT = nc.declare_dram_parameter("wqT", [C, C], F32R, isOutput=False)
    wkT = nc.declare_dram_parameter("wkT", [C, C], F32R, isOutput=False)
    wvT = nc.declare_dram_parameter("wvT", [C, C], F32R, isOutput=False)
    woT = nc.declare_dram_parameter("woT", [C, C], F32R, isOutput=False)
    # bias[:, 0, p] = bq chunk p, bias[:, 1, p] = bk chunk p
    bias = nc.declare_dram_parameter("bias", [128, 2, NP], F32, isOutput=False)
    # bob[q, :] = bo + Wo@bv, replicated over 128 partitions
    bob = nc.declare_dram_parameter("bob", [128, C], F32, isOutput=False)
    ident = nc.declare_dram_parameter("ident", [128, 128], F32, isOutput=False)
    out = nc.declare_dram_parameter("out", [TQ, C], F32, isOutput=True)

    with tile.TileContext(nc) as tc:
        with (
            tc.tile_pool(name="big", bufs=1) as bpool,
            tc.tile_pool(name="const", bufs=1) as cpool,
            tc.tile_pool(name="wgt", bufs=2) as wgt,
            tc.tile_pool(name="ktp", bufs=2) as ktpool,
            tc.tile_pool(name="attnp", bufs=5) as apool,
            tc.tile_pool(name="un", bufs=2) as upool,
            tc.tile_pool(name="ot", bufs=2) as opool,
            tc.tile_pool(name="vb", bufs=1) as vbp,
            tc.tile_pool(name="sc", bufs=3, space="PSUM") as scp,
            tc.tile_pool(name="avp", bufs=1, space="PSUM") as avp,
        ):
            # ---- constants / weights first (small, unblock projections) ----
            bias_s = cpool.tile([128, 2, NP], F32, tag="bias")
            nc.sync.dma_start(bias_s[:], bias[:])
            bob_s = cpool.tile([128, C], F32, tag="bob")
            nc.scalar.dma_start(bob_s[:], bob[:])
            ident_s = cpool.tile([128, 128], F32, tag="ident")
            nc.sync.dma_start(ident_s[:], ident[:])
            ebias = cpool.tile([128, 1], F32, tag="ebias")
            nc.vector.memset(ebias[:], ACT_EXP_BIAS)
            woTs = cpool.tile([128, CT, C], F32R, tag="woT")       # 8KB
            for ct in range(CT):
                nc.scalar.dma_start(woTs[:, ct, :], woT[ct * 128:(ct + 1) * 128, :])
            # x^T resident: one tile per contraction chunk, split across the
            # SP and ACT HWDGE queues so the load is not serialized.
            xTs = []
            for ct in range(CT):
                xt = bpool.tile([128, T], F32R, tag=f"xT{ct}")     # 16KB each
                eng = nc.sync if ct % 2 == 0 else nc.scalar
                eng.dma_start(xt[:, 0:2048], xbT[ct * 128:(ct + 1) * 128, 0:2048])
                eng2 = nc.scalar if ct % 2 == 0 else nc.sync
                eng2.dma_start(xt[:, 2048:T], xbT[ct * 128:(ct + 1) * 128, 2048:T])
                xTs.append(xt)
            # V tiles: per pair-group, 4 chunk tiles of 8 k-tiles each.
            # col 64 of each head slot holds 1.0 (row-sum trick).
            vchunks = [[vbp.tile([128, 8, 4, 65], BF16, tag=f"v{pg}c{c}",
                                 name=f"v{pg}c{c}")
                        for c in range(4)] for pg in range(2)]
            for pg in range(2):
                for c in range(4):
                    nc.vector.memset(vchunks[pg][c][:], 1.0)
            acat = bpool.tile([128, NP, TQ], F32R, tag="acat")     # 16KB

            wvs_tiles = {}

            def load_wvs(pg):
                wvs = wgt.tile([128, CT, 256], F32R, tag="wvs")
                for ct in range(CT):
                    nc.scalar.dma_start(
                        wvs[:, ct, :],
                        wvT[ct * 128:(ct + 1) * 128, pg * 256:(pg + 1) * 256])
                wvs_tiles[pg] = wvs

            def v_step(pg, jv):
                """One k-tile of the V projection for pair-group pg."""
                wvs = wvs_tiles[pg]
                pv = scp.tile([128, 1024], F32, tag="sc")
                for ct in range(CT):
                    nc.tensor.matmul(
                        pv[:, 0:256],
                        xTs[ct][:, jv * 128:(jv + 1) * 128],
                        wvs[:, ct, :],
                        start=(ct == 0), stop=(ct == CT - 1))
                nc.vector.tensor_copy(
                    vchunks[pg][jv // 8][:, jv % 8, :, 0:64],
                    pv[:, 0:256].rearrange("p (a b) -> p a b", b=64))

            def projs(p):
                # --- K^T projection: [128 d, 4096 t], bias bk per-partition ---
                kT = ktpool.tile([128, T], F32R, tag="kT")
                wks = wgt.tile([128, CT, 128], F32R, tag="wks")
                for ct in range(CT):
                    nc.sync.dma_start(
                        wks[:, ct, :],
                        wkT[ct * 128:(ct + 1) * 128, p * 128:(p + 1) * 128])
                for tp in range(4):
                    pk = scp.tile([128, 1024], F32, tag="sc")
                    for half in range(2):
                        t0 = tp * 1024 + half * 512
                        for ct in range(CT):
                            nc.tensor.matmul(
                                pk[:, half * 512:(half + 1) * 512],
                                wks[:, ct, :],
                                xTs[ct][:, t0:t0 + 512],
                                start=(ct == 0), stop=(ct == CT - 1))
                    nc.scalar.activation(kT[:, tp * 1024:(tp + 1) * 1024], pk[:],
                                         AF.Identity, bias=bias_s[:, 1, p:p + 1])
                # --- Q^T projection: [128 d, 1024 q], bias bq (no 1/8 here;
                # the score scale is folded into the exp) ---
                qT = ktpool.tile([128, TQ], F32R, tag="qT")
                wqs = wgt.tile([128, CT, 128], F32R, tag="wqs")
                for ct in range(CT):
                    nc.sync.dma_start(
                        wqs[:, ct, :],
                        wqT[ct * 128:(ct + 1) * 128, p * 128:(p + 1) * 128])
                pq = scp.tile([128, 1024], F32, tag="sc")
                for half in range(2):
                    for ct in range(CT):
                        nc.tensor.matmul(
                            pq[:, half * 512:(half + 1) * 512],
                            wqs[:, ct, :],
                            xTs[ct][:, half * 512:half * 512 + 512],
                            start=(ct == 0), stop=(ct == CT - 1))
                nc.scalar.activation(qT[:], pq[:],
                                     AF.Identity, bias=bias_s[:, 0, p:p + 1])
                return kT, qT

            exp_idx = [0]

            def attn_head(p, h, kT, qT, extra=None):
                hs = (p % 2) * 2 + h
                d0 = h * 64
                # one accumulator spanning exactly 2 PSUM banks; 4 qb regions
                # per bank (qb regions must not straddle the 2KB bank
                # boundary, and start_tensor_calc zeroes a whole bank, so
                # only the first qb of each bank carries start=True).
                av = avp.tile([128, 2, 512], F32, tag="av")

                def emit_scores(j):
                    pss = scp.tile([128, 1024], F32, tag="sc")
                    for half in range(2):
                        nc.tensor.matmul(
                            pss[:, half * 512:(half + 1) * 512],
                            kT[d0:d0 + 64, j * 128:(j + 1) * 128],
                            qT[d0:d0 + 64, half * 512:(half + 1) * 512],
                            start=True, stop=True)
                    return pss

                def emit_exp(pss):
                    at = apool.tile([128, TQ], BF16, tag="attn")
                    if DVE_EXP_ON and exp_idx[0] % 5 in (0, 2):
                        nc.vector._custom_dve(
                            EXP_OP, out=at[:], in0=pss[:],
                            s0=float(EXP_B3), s1=float(EXP_B2),
                            imm2=float(EXP_B1))
                    else:
                        nc.scalar.activation(at[:], pss[:], AF.Exp,
                                             bias=ebias[:], scale=0.125)
                    exp_idx[0] += 1
                    return at

                # software pipeline: scores run 3 k-tiles ahead of attn@V so
                # the in-order PE stream never stalls on an exp.
                pend = [emit_exp(emit_scores(jj)) for jj in range(3)]
                for j in range(KT):
                    if extra is not None:
                        extra(j)
                    if j + 3 < KT:
                        pend.append(emit_exp(emit_scores(j + 3)))
                    at = pend.pop(0)
                    for qb in range(8):
                        s = (qb % 4) * 65
                        nc.tensor.matmul(
                            av[:, qb // 4, s:s + 65],
                            at[:, qb * 128:(qb + 1) * 128],
                            vchunks[p // 2][j // 8][:, j % 8, hs, :],
                            start=(j == 0 and qb % 4 == 0),
                            stop=(j == KT - 1), skip_group_check=True)
                # rowsum sits at free col 64, per q-partition: normalize here
                rd = upool.tile([128, 8], F32, tag="rd")
                for g in range(2):
                    nc.vector.reciprocal(
                        rd[:, g * 4:(g + 1) * 4],
                        av[:, g, 64:324].rearrange("p (a b) -> p a b", b=65)[:, :, 0])
                U = upool.tile([128, 8, 64], F32, tag="U")
                for qb in range(8):
                    s = (qb % 4) * 65
                    nc.vector.tensor_scalar(
                        U[:, qb, :], av[:, qb // 4, s:s + 64],
                        rd[:, qb:qb + 1], None, op0=MULT)
                return U

            def fold_pair(p, U0, U1):
                # transpose U[128q, 64d] tiles back to [64d, 128q] and pack
                # acat[:, p, :] (pair dims on partitions: h0 -> 0:64, h1 -> 64:128).
                # Transpose outputs must sit at PSUM partition 0, so the odd
                # head goes through SBUF and a partition-shifting DMA.
                for h, U in ((0, U0), (1, U1)):
                    pt = scp.tile([128, 1024], F32, tag="sc")
                    for qb in range(8):
                        nc.tensor.matmul(
                            pt[0:64, qb * 128:(qb + 1) * 128],
                            U[:, qb, :], ident_s[:],
                            is_transpose=True, start=True, stop=True)
                    if h == 0:
                        nc.vector.tensor_copy(acat[0:64, p, :], pt[0:64, :])
                    else:
                        odd = opool.tile([64, TQ], F32R, tag="odd")
                        nc.vector.tensor_copy(odd[:], pt[0:64, :])
                        nc.sync.dma_start(acat[64:128, p, :], odd[:])

            # V for pair-group 0: prime VLOOK k-tiles, stream the rest inside
            # pair 0 / head 0. pair-group 1 streams inside pair 1 / head 0.
            load_wvs(0)
            for jv in range(VLOOK):
                v_step(0, jv)

            def extra_pg0(j):
                if j + VLOOK < KT:
                    v_step(0, j + VLOOK)

            def extra_pg1(j):
                if j == 0:
                    load_wvs(1)
                v_step(1, j)

            for p in range(NP):
                kT, qT = projs(p)
                extra = extra_pg0 if p == 0 else (extra_pg1 if p == 1 else None)
                U0 = attn_head(p, 0, kT, qT, extra=extra)
                U1 = attn_head(p, 1, kT, qT)
                fold_pair(p, U0, U1)

            # ---- output projection: out[t, :] = acat^T.T @ woT + bob ----
            for qt in range(8):
                po = scp.tile([128, 1024], F32, tag="sc")
                for r in range(CT):
                    nc.tensor.matmul(
                        po[:, 0:512],
                        acat[:, r, qt * 128:(qt + 1) * 128],
                        woTs[:, r, :],
                        start=(r == 0), stop=(r == CT - 1))
                ot = opool.tile([128, C], F32, tag="ot")
                nc.vector.tensor_tensor(out=ot[:], in0=po[:, 0:512], in1=bob_s[:],
                                        op=ADD)
                nc.sync.dma_start(out[qt * 128:(qt + 1) * 128, :], ot[:])

    nc.compile()
    return nc


def _prep_inputs(x, Wq, bq, Wk, bk, Wv, bv, Wo, bo):
    wqT = np.ascontiguousarray(Wq.T)
    wkT = np.ascontiguousarray(Wk.T)
    wvT = np.ascontiguousarray(Wv.T)
    woT = np.ascontiguousarray(Wo.T)
    bias = np.stack([
        bq.reshape(NP, 128).T,
        bk.reshape(NP, 128).T,
    ], axis=1).astype(np.float32)          # [128, 2, NP]
    bias = np.ascontiguousarray(bias)
    bob = np.ascontiguousarray(
        np.broadcast_to((bo + Wo @ bv).astype(np.float32), (128, C)))
    ident = np.eye(128, dtype=np.float32)
    in_maps = []
    for i in range(8):
        b, q0 = i // 4, (i % 4) * TQ
        xbT = np.ascontiguousarray(np.roll(x[b].T, -q0, axis=1))
        in_maps.append({
            "xbT": xbT, "wqT": wqT, "wkT": wkT, "wvT": wvT, "woT": woT,
            "bias": bias, "bob": bob, "ident": ident,
        })
    return in_maps


def kernel(x, Wq, bq, Wk, bk, Wv, bv, Wo, bo):
    x = np.asarray(x, np.float32)
    args = [np.asarray(a, np.float32) for a in
            (Wq, bq, Wk, bk, Wv, bv, Wo, bo)]
    if "nc" not in _cache:
        _cache["nc"] = _build()
    nc = _cache["nc"]
    in_maps = _prep_inputs(x, *args)
    res = run_bass_kernel_spmd(nc, in_maps, list(range(8)))
    outf = np.empty((B, T, C), np.float32)
    for i in range(8):
        b, q0 = i // 4, (i % 4) * TQ
        outf[b, q0:q0 + TQ, :] = res.results[i]["out"]
    return outf


# revision 44
# speedup vs baseline: 1.2407x; 1.0248x over previous
"""Multi-head attention (B=2, T=4096, D=512, H=8) on 8 Trainium2 cores.

Sharding: core i handles batch b=i//4, query rows q0=(i%4)*1024 .. q0+1024,
all 8 heads (full K/V of its batch computed on-core; no collectives).
Host pre-transposes x and weights so every DMA is contiguous, and rolls
x along T per core so each core's query block sits at columns 0:1024.

Projections run in float32r (TF32-like single-pass PE mode). Attention:
scores^T [128k, 1024q] in f32r; exp split between the Activation engine
(table Exp) and a custom-DVE polynomial (P3(s/32))^4 ~ c*e^(s/8) (softmax
cancels c; ACT tiles carry a matching bias) with output in bf16; attn@V
runs with q on PSUM partitions (out [128q, 65], bf16 moving free 65) so
the ones-column row-sum is a per-partition scalar: normalize with
reciprocal + tensor_scalar, PE-transpose back, and O-project with bv
folded into bo on the host (out = A@Wo^T + (bo + Wo@bv)).

All PSUM [128,1024] f32 tiles (scores, projections, transposes) share one
3-buf pool; V-projection k-tiles are emitted inside the attention loops
(lookahead 10) so the exp pipeline never starves. The scores pipeline
runs 3 k-tiles ahead of attn@V so the in-order PE never waits on an exp."""
import sys
sys.path.insert(0, "/opt/trn_rl_repo")

import math
import numpy as np
import ml_dtypes

import concourse.bacc as bacc
import concourse.mybir as mybir
import concourse.tile as tile
import concourse.dve_ops as dops
from concourse.dve_spec import Spec, Bin, AluOp, Src0, C0, C1, C2, One, sq, lower
from concourse.dve_uop import DveOpSpec
from concourse.bass_utils import run_bass_kernel_spmd

F32 = mybir.dt.float32
F32R = mybir.dt.float32r
BF16 = mybir.dt.bfloat16
AF = mybir.ActivationFunctionType
MULT = mybir.AluOpType.mult
ADD = mybir.AluOpType.add

B, T, C = 2, 4096, 512
H, DK = 8, 64
TQ = 1024          # queries per core
NP = 4             # head pairs
KT = T // 128      # 32 k-tiles
CT = C // 128      # 4 contraction tiles
VLOOK = 10         # v-proj k-tiles emitted this far ahead of attn@V use

# deg-3 fit of e^u on [-0.57, 0.57]; P/a0 has unit constant term and
# approximates (1/a0)*e^u. (P/a0)^4 = c*e^(4u); softmax cancels c, the
# ACT-exp tiles get bias=-4*ln(a0) so both engines agree.
_A0, _A1, _A2, _A3 = 0.99951510, 1.00118603, 0.51320528, 0.16396123
EXP_B1 = (_A1 / _A0) / 32.0
EXP_B2 = (_A2 / _A0) / 32.0**2
EXP_B3 = (_A3 / _A0) / 32.0**3
ACT_EXP_BIAS = -4.0 * math.log(_A0)
DVE_EXP_ON = True


def _register_exp_op():
    name = "EXP_POLY34_ANT"
    if name in dops._SUB_OPCODE_FOR_NAME:
        return next(o for o in dops.OPS if o.name == name)
    body = sq(sq(Bin(AluOp.ADD,
                     Bin(AluOp.MULTIPLY,
                         Bin(AluOp.ADD,
                             Bin(AluOp.MULTIPLY,
                                 Bin(AluOp.ADD, Bin(AluOp.MULTIPLY, C0, Src0), C1),
                                 Src0),
                             C2),
                         Src0),
                     One)))

    def ref(in0, in1, s0, s1, imm2):
        x = in0.astype(np.float32)
        p = ((s0 * x + s1) * x + imm2) * x + 1.0
        return (p * p) * (p * p)

    spec = Spec(body=body, reference=ref)
    row = dops._CUSTOM_DVE_ROW_BASE + len(dops.OPS)
    assert row < 0x20
    dops._SUB_OPCODE_FOR_NAME[name] = row
    sha = DveOpSpec(name=name, opcode=row, uops=lower(spec, ver="v3"),
                    rd1_en=False).sha("v3")
    op = dops.DveOp(name=name, spec=spec, subdim=False, uops_sha={"v3": sha})
    dops.OPS.append(op)
    dops.CUSTOM_DVE_SPECS[name] = spec
    return op


EXP_OP = _register_exp_op()
_cache = {}


def _build():
    nc = bacc.Bacc("TRN2")
    xbT = nc.declare_dram_parameter("xbT", [C, T], BF16, isOutput=False)
    # pre-tiled weights (one DMA each): wkq[p][:, ct, 0, :] = Wk^T chunk,
    # [..., 1, :] = Wq^T chunk; wv2[pg][:, ct, :] = Wv^T cols; wot[:, ct, :].
    wkq = nc.declare_dram_parameter("wkq", [NP, 128, CT, 2, 128], BF16,
                                    isOutput=False)
    wv2 = nc.declare_dram_parameter("wv2", [2, 128, CT, 256], BF16,
                                    isOutput=False)
    wot = nc.declare_dram_parameter("wot", [128, CT, C], F32R, isOutput=False)
    # consts[:, 0:8] = biases (bq|bk per pair), 8:136 = identity,
    # 136:648 = bo + Wo@bv replicated over partitions
    consts = nc.declare_dram_parameter("consts", [128, 648], F32,
                                       isOutput=False)
    out = nc.declare_dram_parameter("out", [TQ, C], F32, isOutput=True)

    with tile.TileContext(nc) as tc:
        with (
            tc.tile_pool(name="big", bufs=1) as bpool,
            tc.tile_pool(name="const", bufs=1) as cpool,
            tc.tile_pool(name="wgt", bufs=2) as wgt,
            tc.tile_pool(name="ktp", bufs=2) as ktpool,
            tc.tile_pool(name="attnp", bufs=5) as apool,
            tc.tile_pool(name="un", bufs=2) as upool,
            tc.tile_pool(name="ot", bufs=2) as opool,
            tc.tile_pool(name="vb", bufs=1) as vbp,
            tc.tile_pool(name="sc", bufs=3, space="PSUM") as scp,
            tc.tile_pool(name="avp", bufs=1, space="PSUM") as avp,
        ):
            # ---- constants / weights first (small, unblock projections) ----
            consts_s = cpool.tile([128, 648], F32, tag="consts")
            nc.sync.dma_start(consts_s[:], consts[:])
            bias_s = consts_s[:, 0:8].rearrange("p (a b) -> p a b", b=NP)
            ident_s = consts_s[:, 8:136]
            bob_s = consts_s[:, 136:648]
            ebias = cpool.tile([128, 1], F32, tag="ebias")
            nc.vector.memset(ebias[:], ACT_EXP_BIAS)
            # pair-0 K/Q weights and both wvs jump the queue ahead of the
            # big x loads so the first projections start early.
            wgt_kq = {}

            def load_kq_weights(p):
                wkqs = wgt.tile([128, CT, 2, 128], BF16, tag="wkq", name="wkq")
                nc.sync.dma_start(wkqs[:], wkq[p])
                wgt_kq[p] = wkqs

            load_kq_weights(0)
            woTs = cpool.tile([128, CT, C], F32R, tag="woT")       # 8KB
            nc.scalar.dma_start(woTs[:], wot[:])
            # x^T resident: one tile per (contraction chunk, T-half), split
            # across the SP and ACT HWDGE queues so the load is not serialized
            # and the first K-proj chunks only wait for the first T-half.
            xts2 = [[bpool.tile([128, 2048], BF16, tag=f"xT{ct}h{hf}",
                                name=f"xT{ct}h{hf}")
                     for hf in range(2)] for ct in range(CT)]
            for hf in range(2):          # first T-half of every ct first
                for ct in range(CT):
                    eng = nc.sync if (ct + hf) % 2 == 0 else nc.scalar
                    eng.dma_start(xts2[ct][hf][:],
                                  xbT[ct * 128:(ct + 1) * 128,
                                      hf * 2048:(hf + 1) * 2048])

            def xT(ct, t0, width):
                hf = t0 // 2048
                assert (t0 + width - 1) // 2048 == hf
                return xts2[ct][hf][:, t0 - hf * 2048:t0 - hf * 2048 + width]
            # V tiles: per pair-group, 4 chunk tiles of 8 k-tiles each.
            # col 64 of each head slot holds 1.0 (row-sum trick).
            vchunks = [[vbp.tile([128, 8, 4, 65], BF16, tag=f"v{pg}c{c}",
                                 name=f"v{pg}c{c}")
                        for c in range(4)] for pg in range(2)]
            for pg in range(2):
                for c in range(4):
                    # only the ones-column needs initializing
                    nc.vector.memset(vchunks[pg][c][:, :, :, 64], 1.0)
            acat = bpool.tile([128, NP, TQ], F32R, tag="acat")     # 16KB

            wvs_tiles = {}

            def load_wvs(pg):
                wvs = wgt.tile([128, CT, 256], BF16, tag="wvs", name="wvs")
                nc.scalar.dma_start(wvs[:], wv2[pg])
                wvs_tiles[pg] = wvs

            def v_step(pg, jv):
                """One k-tile of the V projection for pair-group pg."""
                wvs = wvs_tiles[pg]
                pv = scp.tile([128, 1024], F32, tag="sc")
                for ct in range(CT):
                    nc.tensor.matmul(
                        pv[:, 0:256],
                        xT(ct, jv * 128, 128),
                        wvs[:, ct, :],
                        start=(ct == 0), stop=(ct == CT - 1))
                nc.vector.tensor_copy(
                    vchunks[pg][jv // 8][:, jv % 8, :, 0:64],
                    pv[:, 0:256].rearrange("p (a b) -> p a b", b=64))

            def projs(p):
                # --- K^T projection: [128 d, 4096 t], bias bk per-partition ---
                kT = ktpool.tile([128, T], F32R, tag="kT")
                wkqs = wgt_kq.pop(p)
                wks = wkqs[:, :, 0, :]
                wqs = wkqs[:, :, 1, :]
                for tp in range(4):
                    pk = scp.tile([128, 1024], F32, tag="sc")
                    for half in range(2):
                        t0 = tp * 1024 + half * 512
                        for ct in range(CT):
                            nc.tensor.matmul(
                                pk[:, half * 512:(half + 1) * 512],
                                wks[:, ct, :],
                                xT(ct, t0, 512),
                                start=(ct == 0), stop=(ct == CT - 1))
                    nc.scalar.activation(kT[:, tp * 1024:(tp + 1) * 1024], pk[:],
                                         AF.Identity, bias=bias_s[:, 1, p:p + 1])
                # --- Q^T projection: [128 d, 1024 q], bias bq (no 1/8 here;
                # the score scale is folded into the exp) ---
                qT = ktpool.tile([128, TQ], F32R, tag="qT")
                pq = scp.tile([128, 1024], F32, tag="sc")
                for half in range(2):
                    for ct in range(CT):
                        nc.tensor.matmul(
                            pq[:, half * 512:(half + 1) * 512],
                            wqs[:, ct, :],
                            xT(ct, half * 512, 512),
                            start=(ct == 0), stop=(ct == CT - 1))
                nc.scalar.activation(qT[:], pq[:],
                                     AF.Identity, bias=bias_s[:, 0, p:p + 1])
                return kT, qT

            exp_idx = [0]

            def attn_head(p, h, kT, qT, extra=None):
                hs = (p % 2) * 2 + h
                d0 = h * 64
                # one accumulator spanning exactly 2 PSUM banks; 4 qb regions
                # per bank (qb regions must not straddle the 2KB bank
                # boundary, and start_tensor_calc zeroes a whole bank, so
                # only the first qb of each bank carries start=True).
                av = avp.tile([128, 2, 512], F32, tag="av")

                def emit_scores(j):
                    pss = scp.tile([128, 1024], F32, tag="sc")
                    for half in range(2):
                        nc.tensor.matmul(
                            pss[:, half * 512:(half + 1) * 512],
                            kT[d0:d0 + 64, j * 128:(j + 1) * 128],
                            qT[d0:d0 + 64, half * 512:(half + 1) * 512],
                            start=True, stop=True)
                    return pss

                def emit_exp(pss):
                    at = apool.tile([128, TQ], BF16, tag="attn")
                    if DVE_EXP_ON and exp_idx[0] % 5 in (0, 2):
                        nc.vector._custom_dve(
                            EXP_OP, out=at[:], in0=pss[:],
                            s0=float(EXP_B3), s1=float(EXP_B2),
                            imm2=float(EXP_B1))
                    else:
                        nc.scalar.activation(at[:], pss[:], AF.Exp,
                                             bias=ebias[:], scale=0.125)
                    exp_idx[0] += 1
                    return at

                # software pipeline: scores run 3 k-tiles ahead of attn@V so
                # the in-order PE stream never stalls on an exp.
                pend = [emit_exp(emit_scores(jj)) for jj in range(3)]
                for j in range(KT):
                    if extra is not None:
                        extra(j)
                    if j + 3 < KT:
                        pend.append(emit_exp(emit_scores(j + 3)))
                    at = pend.pop(0)
                    for qb in range(8):
                        s = (qb % 4) * 65
                        nc.tensor.matmul(
                            av[:, qb // 4, s:s + 65],
                            at[:, qb * 128:(qb + 1) * 128],
                            vchunks[p // 2][j // 8][:, j % 8, hs, :],
                            start=(j == 0 and qb % 4 == 0),
                            stop=(j == KT - 1), skip_group_check=True)
                # rowsum sits at free col 64, per q-partition: normalize here
                rd = upool.tile([128, 8], F32, tag="rd")
                for g in range(2):
                    nc.vector.reciprocal(
                        rd[:, g * 4:(g + 1) * 4],
                        av[:, g, 64:324].rearrange("p (a b) -> p a b", b=65)[:, :, 0])
                U = upool.tile([128, 8, 64], F32, tag="U")
                for qb in range(8):
                    s = (qb % 4) * 65
                    nc.vector.tensor_scalar(
                        U[:, qb, :], av[:, qb // 4, s:s + 64],
                        rd[:, qb:qb + 1], None, op0=MULT)
                return U

            def fold_pair(p, U0, U1):
                # transpose U[128q, 64d] tiles back to [64d, 128q] and pack
                # acat[:, p, :] (pair dims on partitions: h0 -> 0:64, h1 -> 64:128).
                # Transpose outputs must sit at PSUM partition 0, so the odd
                # head goes through SBUF and a partition-shifting DMA.
                for h, U in ((0, U0), (1, U1)):
                    pt = scp.tile([128, 1024], F32, tag="sc")
                    for qb in range(8):
                        nc.tensor.matmul(
                            pt[0:64, qb * 128:(qb + 1) * 128],
                            U[:, qb, :], ident_s[:],
                            is_transpose=True, start=True, stop=True)
                    if h == 0:
                        nc.vector.tensor_copy(acat[0:64, p, :], pt[0:64, :])
                    else:
                        odd = opool.tile([64, TQ], F32R, tag="odd", bufs=1)
                        nc.vector.tensor_copy(odd[:], pt[0:64, :])
                        nc.sync.dma_start(acat[64:128, p, :], odd[:])

            # V for pair-group 0: prime VLOOK k-tiles, stream the rest inside
            # pair 0 / head 0. pair-group 1 streams inside pair 1 / head 0.
            load_wvs(0)
            load_wvs(1)
            for jv in range(VLOOK):
                v_step(0, jv)

            def extra_pg0(j):
                if j + VLOOK < KT:
                    v_step(0, j + VLOOK)

            def extra_pg1(j):
                v_step(1, j)

            for p in range(NP):
                kT, qT = projs(p)
                if p + 1 < NP:
                    load_kq_weights(p + 1)
                extra = extra_pg0 if p == 0 else (extra_pg1 if p == 1 else None)
                U0 = attn_head(p, 0, kT, qT, extra=extra)
                U1 = attn_head(p, 1, kT, qT)
                fold_pair(p, U0, U1)

            # ---- output projection: out[t, :] = acat^T.T @ woT + bob ----
            for qt in range(8):
                po = scp.tile([128, 1024], F32, tag="sc")
                for r in range(CT):
                    nc.tensor.matmul(
                        po[:, 0:512],
                        acat[:, r, qt * 128:(qt + 1) * 128],
                        woTs[:, r, :],
                        start=(r == 0), stop=(r == CT - 1))
                if qt % 2 == 0:
                    ot = opool.tile([128, 2, C], F32, tag="ot", bufs=1)
                nc.vector.tensor_tensor(out=ot[:, qt % 2, :], in0=po[:, 0:512],
                                        in1=bob_s, op=ADD)
                if qt % 2 == 1:
                    eng = nc.sync if qt % 4 == 1 else nc.scalar
                    dst = out[(qt - 1) * 128:(qt + 1) * 128, :].rearrange(
                        "(s p) c -> p s c", s=2)
                    eng.dma_start(dst, ot[:])

    nc.compile()
    return nc


def _prep_inputs(x, Wq, bq, Wk, bk, Wv, bv, Wo, bo):
    wqT = Wq.T
    wkT = Wk.T
    wvT = Wv.T
    woT = Wo.T
    wkq = np.empty((NP, 128, CT, 2, 128), ml_dtypes.bfloat16)
    wv2 = np.empty((2, 128, CT, 256), ml_dtypes.bfloat16)
    wot = np.empty((128, CT, C), np.float32)
    for ct in range(CT):
        rows = slice(ct * 128, (ct + 1) * 128)
        for p in range(NP):
            wkq[p, :, ct, 0, :] = wkT[rows, p * 128:(p + 1) * 128]
            wkq[p, :, ct, 1, :] = wqT[rows, p * 128:(p + 1) * 128]
        for pg in range(2):
            wv2[pg, :, ct, :] = wvT[rows, pg * 256:(pg + 1) * 256]
        wot[:, ct, :] = woT[rows, :]
    consts = np.empty((128, 648), np.float32)
    consts[:, 0:4] = bq.reshape(NP, 128).T
    consts[:, 4:8] = bk.reshape(NP, 128).T
    consts[:, 8:136] = np.eye(128, dtype=np.float32)
    consts[:, 136:648] = (bo + Wo @ bv).astype(np.float32)[None, :]
    in_maps = []
    for i in range(8):
        b, q0 = i // 4, (i % 4) * TQ
        xbT = np.ascontiguousarray(
            np.roll(x[b].T, -q0, axis=1).astype(ml_dtypes.bfloat16))
        in_maps.append({
            "xbT": xbT, "wkq": wkq, "wv2": wv2, "wot": wot, "consts": consts,
        })
    return in_maps


def kernel(x, Wq, bq, Wk, bk, Wv, bv, Wo, bo):
    x = np.asarray(x, np.float32)
    args = [np.asarray(a, np.float32) for a in
            (Wq, bq, Wk, bk, Wv, bv, Wo, bo)]
    if "nc" not in _cache:
        _cache["nc"] = _build()
    nc = _cache["nc"]
    in_maps = _prep_inputs(x, *args)
    res = run_bass_kernel_spmd(nc, in_maps, list(range(8)))
    outf = np.empty((B, T, C), np.float32)
    for i in range(8):
        b, q0 = i // 4, (i % 4) * TQ
        outf[b, q0:q0 + TQ, :] = res.results[i]["out"]
    return outf
